# revision 18
# baseline (speedup 1.0000x reference)
"""Trainium2 Bass kernel for nn_AMFAR (retrieval_knn, 8 NeuronCores).

Strategy: data-parallel over the 65536 queries (8192/core). Host pre-transposes
bf16 queries to [D, Qs] (matmul needs the contraction dim on partitions and
f32 DMA-transpose doesn't exist), precomputes |q|^2 / |p|^2 from the rounded
values and scales protos by -2 so PSUM accumulates the full squared distance.
Device computes sqrt via exp(0.5*ln(x)) to stay inside the one ACT table set
that has both exp and ln. Scalar losses are finished on host from per-query
device outputs (the global n_f cutoff needs a global mask count anyway).
"""
import os
import sys

sys.path.insert(0, "/opt/trn_rl_repo")

import numpy as np
import ml_dtypes

import concourse.bass as bass
import concourse.bacc as bacc
import concourse.tile as tile
from concourse import mybir
from concourse.bass_utils import run_bass_kernel_spmd

BF16 = mybir.dt.bfloat16
F32 = mybir.dt.float32
AF = mybir.ActivationFunctionType
ALU = mybir.AluOpType
AX = mybir.AxisListType

N_CORES = 8
Q, C, D = 65536, 64, 512
QS = Q // N_CORES          # 8192 queries per core
QBLK = 1024                # queries per supertile
NSUP = QS // QBLK          # 8
NT = QBLK // 128           # 8 q-subtiles per supertile
NTILES = QS // 128         # 64 q-subtiles per core
HN = NTILES // 2           # half, for split B/C phases
KCH = D // 128             # 4 contraction chunks
AUGK = 4                   # augmentation rows (q2_hi/q2_lo/1/1), bf16 hi-lo split
MODS = ("r", "f")

LAST_RESULT = None         # BassKernelResults of the last run (for test.py)


def _install_ntff_hook():
    """The agent image's antenv lacks axon_hooks; provide the shim so
    run_bass_kernel_spmd(trace=True) can NTFF-profile through the tunnel."""
    import types

    if "antenv.axon_hooks" in sys.modules:
        return
    mod = types.ModuleType("antenv.axon_hooks")
    mod._hook = None

    def set_axon_ntff_profile_hook(h):
        mod._hook = h

    def get_axon_ntff_profile_hook():
        return mod._hook

    mod.set_axon_ntff_profile_hook = set_axon_ntff_profile_hook
    mod.get_axon_ntff_profile_hook = get_axon_ntff_profile_hook
    sys.modules["antenv.axon_hooks"] = mod
    try:
        from trn_agent_boot.trn_boot import _ntff_profile_via_ctypes

        hook = _ntff_profile_via_ctypes("/opt/axon/libaxon_pjrt.so")
        if hook is not None:
            mod._hook = hook
    except Exception:
        pass


_install_ntff_hook()


def _patch_act_tables():
    """Make natural_log_exp_and_others the only set holding Exp and Ln, so
    bacc's table-load pass emits ONE load instead of thrashing between the
    exp-only and ln-only sets every supertile (33 loads x 1.5us measured)."""
    import functools
    import concourse.hw_specs as hw_specs

    orig = hw_specs.get_activation_tables

    @functools.cache
    def patched(arch):
        out = {}
        for name, fns in orig(arch).items():
            if name != "natural_log_exp_and_others":
                fns = set(fns) - {AF.Exp, AF.Ln}
            out[name] = set(fns)
        return out

    bacc.get_activation_tables = patched


_patch_act_tables()



def _bcast(ap, n):
    """Broadcast an AP along a new innermost free dim of size n (stride 0)."""
    return bass.AP(tensor=ap.tensor, offset=ap.offset, ap=[*ap.ap, [0, n]])


def _flat(ap):
    return ap.rearrange("p a b -> p (a b)")


def _build():
    nc = bacc.Bacc(None, target_bir_lowering=False)
    I, O = {}, {}
    for m in MODS:
        I[f"qT_{m}"] = nc.declare_dram_parameter(f"qT_{m}", [D, QS], BF16, isOutput=False)
        I[f"q2_{m}"] = nc.declare_dram_parameter(f"q2_{m}", [NT * AUGK, NSUP * 128], BF16, isOutput=False)
        I[f"pT_{m}"] = nc.declare_dram_parameter(f"pT_{m}", [128, KCH * C], BF16, isOutput=False)
        I[f"p2_{m}"] = nc.declare_dram_parameter(f"p2_{m}", [NT * AUGK, NT * C], BF16, isOutput=False)
        O[f"post_{m}"] = nc.declare_dram_parameter(f"post_{m}", [128, NTILES, C], BF16, isOutput=True)
        O[f"c_{m}"] = nc.declare_dram_parameter(f"c_{m}", [128, NTILES], F32, isOutput=True)
        O[f"w_{m}"] = nc.declare_dram_parameter(f"w_{m}", [128, NTILES], F32, isOutput=True)
    O["fused"] = nc.declare_dram_parameter("fused", [128, NTILES, C], BF16, isOutput=True)
    O["mask"] = nc.declare_dram_parameter("mask", [128, NTILES], F32, isOutput=True)

    with tile.TileContext(nc) as tc:
        _body(nc, tc, I, O)
    nc.compile()
    return nc


def _body(nc, tc, I, O):
    from contextlib import ExitStack

    with ExitStack() as ctx:
        const = ctx.enter_context(tc.tile_pool(name="const", bufs=1))
        qpool = ctx.enter_context(tc.tile_pool(name="qpool", bufs=3))
        psum = ctx.enter_context(tc.tile_pool(name="psum", bufs=3, space="PSUM"))
        work = ctx.enter_context(tc.tile_pool(name="work", bufs=3))
        outp = ctx.enter_context(tc.tile_pool(name="outp", bufs=3))
        stat = ctx.enter_context(tc.tile_pool(name="stat", bufs=1))
        small = ctx.enter_context(tc.tile_pool(name="small", bufs=2))

        pT, p2 = {}, {}
        for m in MODS:
            pT[m] = const.tile([128, KCH * C], BF16, tag=f"pT_{m}", name=f"pT_{m}")
            nc.sync.dma_start(out=pT[m][:, :], in_=I[f"pT_{m}"][:, :])
            p2[m] = const.tile([NT * AUGK, NT * C], BF16, tag=f"p2_{m}", name=f"p2_{m}")
            nc.sync.dma_start(out=p2[m][:, :], in_=I[f"p2_{m}"][:, :])

        q2all = {}
        for m in MODS:
            q2all[m] = const.tile([NT * AUGK, NSUP * 128], BF16, tag=f"q2a_{m}", name=f"q2a_{m}")
            nc.sync.dma_start(out=q2all[m][:, :], in_=I[f"q2_{m}"][:, :])

        # persistent per-query staging, [128, NTILES(=64)] each
        st = {}
        for nm in ("S_r", "S_f", "rS_r", "rS_f", "ce_r", "ce_f", "pde_r", "pde_f",
                   "Xe", "c_r", "c_f", "w_r", "w_f", "mask", "A"):
            st[nm] = stat.tile([128, NTILES], F32, tag=f"st_{nm}", name=f"st_{nm}")
        # posteriors staged for the whole core (bf16): [128, NTILES, C]
        pall = {}
        for m in MODS:
            pall[m] = stat.tile([128, NTILES, C], BF16, tag=f"pall_{m}", name=f"pall_{m}")

        # ---- phase B+C per half: batched scalar math + fused posterior ----
        def phase_bc(hsl, tag):
            h = {}
            for m in MODS:
                nc.vector.tensor_mul(st[f"c_{m}"][:, hsl], st[f"ce_{m}"][:, hsl], st[f"rS_{m}"][:, hsl])
                lnS = small.tile([128, HN], F32, tag=f"lnS_{m}{tag}", name=f"lnS_{m}{tag}")
                nc.scalar.activation(lnS[:, :], st[f"S_{m}"][:, hsl], AF.Ln)
                h[m] = small.tile([128, HN], F32, tag=f"h_{m}{tag}", name=f"h_{m}{tag}")
                nc.vector.tensor_mul(h[m][:, :], st[f"pde_{m}"][:, hsl], st[f"rS_{m}"][:, hsl])
                nc.vector.tensor_add(h[m][:, :], h[m][:, :], lnS[:, :])

            X = small.tile([128, HN], F32, tag=f"X{tag}", name=f"X{tag}")
            nc.vector.tensor_mul(X[:, :], st["Xe"][:, hsl], st["rS_r"][:, hsl])
            nc.vector.tensor_mul(X[:, :], X[:, :], st["rS_f"][:, hsl])

            for a_, wname in (("f", "w_f"), ("r", "w_r")):
                kl = small.tile([128, HN], F32, tag=f"kl_{a_}{tag}", name=f"kl_{a_}{tag}")
                nc.vector.tensor_add(kl[:, :], h[a_][:, :], X[:, :])
                nc.vector.tensor_scalar_mul(kl[:, :], kl[:, :], -1.0 / C)
                nc.vector.tensor_mul(st[wname][:, hsl], st[f"c_{a_}"][:, hsl], kl[:, :])

            g1 = small.tile([128, HN], F32, tag=f"g1{tag}", name=f"g1{tag}")
            nc.vector.tensor_tensor(out=g1[:, :], in0=h["r"][:, :], in1=h["f"][:, :], op=ALU.is_gt)
            g2 = small.tile([128, HN], F32, tag=f"g2{tag}", name=f"g2{tag}")
            nc.vector.tensor_tensor(out=g2[:, :], in0=st["c_r"][:, hsl], in1=st["c_f"][:, hsl], op=ALU.is_gt)
            nc.vector.tensor_mul(st["mask"][:, hsl], g1[:, :], g2[:, :])

            den = small.tile([128, HN], F32, tag=f"den{tag}", name=f"den{tag}")
            nc.vector.tensor_add(den[:, :], st["c_r"][:, hsl], st["c_f"][:, hsl])
            rden = small.tile([128, HN], F32, tag=f"rden{tag}", name=f"rden{tag}")
            nc.vector.reciprocal(rden[:, :], den[:, :])
            wr = small.tile([128, HN], F32, tag=f"wr{tag}", name=f"wr{tag}")
            nc.vector.tensor_mul(wr[:, :], st["c_r"][:, hsl], rden[:, :])
            a_t = small.tile([128, HN], F32, tag=f"a_t{tag}", name=f"a_t{tag}")
            nc.vector.tensor_mul(a_t[:, :], wr[:, :], st["S_r"][:, hsl])
            wf = small.tile([128, HN], F32, tag=f"wf{tag}", name=f"wf{tag}")
            nc.vector.tensor_scalar(wf[:, :], wr[:, :], -1.0, 1.0, ALU.mult, ALU.add)
            b_t = small.tile([128, HN], F32, tag=f"b_t{tag}", name=f"b_t{tag}")
            nc.vector.tensor_mul(b_t[:, :], wf[:, :], st["S_f"][:, hsl])
            T_t = small.tile([128, HN], F32, tag=f"T_t{tag}", name=f"T_t{tag}")
            nc.vector.tensor_add(T_t[:, :], a_t[:, :], b_t[:, :])
            rT = small.tile([128, HN], F32, tag=f"rT{tag}", name=f"rT{tag}")
            nc.vector.reciprocal(rT[:, :], T_t[:, :])
            A_t = small.tile([128, HN], BF16, tag=f"A_t{tag}", name=f"A_t{tag}")
            nc.vector.tensor_mul(A_t[:, :], a_t[:, :], rT[:, :])

            # fused = A*(p_r - p_f) + p_f
            for su in range(hsl.start // NT, hsl.stop // NT):
                ssl = slice(su * NT, (su + 1) * NT)
                asl = slice(su * NT - hsl.start, (su + 1) * NT - hsl.start)
                df = work.tile([128, NT, C], BF16, tag="t1", name="t1")
                nc.vector.tensor_sub(df[:, :, :], pall["r"][:, ssl, :], pall["f"][:, ssl, :])
                t2 = work.tile([128, NT, C], BF16, tag="t2", name="t2")
                nc.vector.tensor_tensor(
                    out=t2[:, :, :], in0=df[:, :, :], in1=_bcast(A_t[:, asl], C),
                    op=ALU.mult,
                )
                fu = outp.tile([128, NT, C], BF16, tag="fused", name="fused")
                nc.vector.tensor_add(fu[:, :, :], t2[:, :, :], pall["f"][:, ssl, :])
                nc.scalar.dma_start(out=O["fused"][:, ssl, :], in_=fu[:, :, :])


        # ---- phase A: distances, posteriors, raw stats ----
        for su in range(NSUP):
            q0 = su * QBLK
            ssl = slice(su * NT, (su + 1) * NT)

            qt = {}
            for m in MODS:
                for k in range(KCH):
                    t = qpool.tile([128, QBLK], BF16, tag=f"qt_{m}{k}", name=f"qt_{m}{k}")
                    nc.sync.dma_start(
                        out=t[:, :], in_=I[f"qT_{m}"][k * 128:(k + 1) * 128, q0:q0 + QBLK]
                    )
                    qt[m, k] = t

            ps = {}
            for m in MODS:
                P_ = psum.tile([128, NT, C], F32, tag=f"ps_{m}", name=f"ps_{m}")
                nc.tensor.matmul(
                    _flat(P_),
                    lhsT=q2all[m][:, su * 128:(su + 1) * 128],
                    rhs=p2[m][:, :],
                    start=True,
                    stop=False,
                )
                for t_i in range(NT):
                    sl = slice(t_i * 128, (t_i + 1) * 128)
                    for k in range(KCH):
                        nc.tensor.matmul(
                            P_[:, t_i, :],
                            lhsT=qt[m, k][:, sl],
                            rhs=pT[m][:, k * C:(k + 1) * C],
                            start=False,
                            stop=(k == KCH - 1),
                            skip_group_check=True,
                        )
                ps[m] = P_

            e, d = {}, {}
            for m in MODS:
                lnt = work.tile([128, NT * C], F32, tag=f"ln_{m}", name=f"ln_{m}")
                nc.scalar.activation(lnt[:, :], _flat(ps[m]), AF.Ln)
                d[m] = work.tile([128, NT, C], F32, tag=f"d_{m}", name=f"d_{m}")
                nc.scalar.activation(_flat(d[m]), lnt[:, :], AF.Exp, scale=0.5)
                e[m] = work.tile([128, NT, C], BF16, tag=f"e_{m}", name=f"e_{m}")
                nc.scalar.activation(_flat(e[m]), _flat(d[m]), AF.Exp, scale=-1.0)

                nc.vector.reduce_sum(st[f"S_{m}"][:, ssl], e[m][:, :, :], axis=AX.X)
                nc.vector.reciprocal(st[f"rS_{m}"][:, ssl], st[f"S_{m}"][:, ssl])
                nc.vector.reduce_max(st[f"ce_{m}"][:, ssl], e[m][:, :, :], axis=AX.X)
                rSb = small.tile([128, NT], BF16, tag=f"rSb_{m}", name=f"rSb_{m}")
                nc.vector.tensor_copy(rSb[:, :], st[f"rS_{m}"][:, ssl])
                # posterior p = e * rS  (bf16, staged + output)
                pv = pall[m][:, ssl, :]
                nc.vector.tensor_tensor(
                    out=pv, in0=e[m][:, :, :], in1=_bcast(rSb[:, :], C),
                    op=ALU.mult,
                )
                nc.scalar.dma_start(out=O[f"post_{m}"][:, ssl, :], in_=pv)

                # sum(e*d) for entropy
                pdt = work.tile([128, NT, C], F32, tag="scratch", name="scratch")
                nc.gpsimd.tensor_mul(pdt[:, :, :], e[m][:, :, :], d[m][:, :, :])
                nc.vector.reduce_sum(st[f"pde_{m}"][:, ssl], pdt[:, :, :], axis=AX.X)

            # cross term sum(e_r*e_f)
            xt = work.tile([128, NT, C], BF16, tag="xscr", name="xscr")
            nc.vector.tensor_mul(xt[:, :, :], e["r"][:, :, :], e["f"][:, :, :])
            nc.vector.reduce_sum(st["Xe"][:, ssl], xt[:, :, :], axis=AX.X)

            if su == NSUP // 2 - 1:
                phase_bc(slice(0, NSUP // 2 * NT), "a")
            elif su == NSUP - 1:
                phase_bc(slice(NSUP // 2 * NT, NSUP * NT), "b")

        for nm in ("c_r", "c_f", "w_r", "w_f", "mask"):
            nc.scalar.dma_start(out=O[nm][:, :], in_=st[nm][:, :])


_GRAPH = None


def _graph():
    global _GRAPH
    if _GRAPH is None:
        _GRAPH = _build()
    return _GRAPH


def kernel(context_rgb_features, context_flow_features,
           target_rgb_features, target_flow_features):
    global LAST_RESULT
    bf = ml_dtypes.bfloat16
    ctxf = {"r": np.asarray(context_rgb_features), "f": np.asarray(context_flow_features)}
    tgt = {"r": np.asarray(target_rgb_features), "f": np.asarray(target_flow_features)}

    pT_in, p2_in = {}, {}
    for m in MODS:
        pb = ctxf[m].astype(bf)                                   # [C, D] rounded
        pbf = pb.astype(np.float32)
        pTs = np.ascontiguousarray((-2.0 * pbf).astype(bf).T)     # [D, C] = -2*p~
        pT_in[m] = np.ascontiguousarray(
            pTs.reshape(KCH, 128, C).transpose(1, 0, 2)
        ).reshape(128, KCH * C)
        p2v = (pbf * pbf).sum(1)
        p2_hi = p2v.astype(bf).astype(np.float32)
        p2row = np.zeros((AUGK, C), bf)
        p2row[0] = 1.0                                            # pairs q2_hi row
        p2row[1] = 1.0                                            # pairs q2_lo row
        p2row[2] = p2_hi.astype(bf)                               # pairs ones row
        p2row[3] = (p2v - p2_hi).astype(bf)                       # residual
        # block-diagonal over the NT subtiles: rows (t,j), cols (t,c)
        p2a = np.zeros((NT * AUGK, NT * C), bf)
        for t in range(NT):
            p2a[t * AUGK:(t + 1) * AUGK, t * C:(t + 1) * C] = p2row
        p2_in[m] = p2a

    in_maps = []
    for core in range(N_CORES):
        sl = slice(core * QS, (core + 1) * QS)
        im = {}
        for m in MODS:
            tb = tgt[m][sl].astype(bf)                            # [QS, D] rounded
            tf32 = tb.astype(np.float32)
            im[f"qT_{m}"] = np.ascontiguousarray(tb.T)            # [D, QS]
            q2v = np.einsum("qd,qd->q", tf32, tf32)
            q2_hi = q2v.astype(bf).astype(np.float32)
            q2lo = (q2v - q2_hi)
            # rows (t,j), cols (su,p): value = aug_j(su*QBLK + t*128 + p)
            q2a = np.empty((NT, AUGK, NSUP, 128), bf)
            qq = q2_hi.reshape(NSUP, NT, 128)
            ql = q2lo.reshape(NSUP, NT, 128)
            q2a[:, 0] = qq.transpose(1, 0, 2).astype(bf)
            q2a[:, 1] = ql.transpose(1, 0, 2).astype(bf)
            q2a[:, 2] = 1.0
            q2a[:, 3] = 1.0
            im[f"q2_{m}"] = q2a.reshape(NT * AUGK, NSUP * 128)
            im[f"pT_{m}"] = pT_in[m]
            im[f"p2_{m}"] = p2_in[m]
        in_maps.append(im)

    trace = os.environ.get("KERNEL_TRACE", "0") == "1"
    res = run_bass_kernel_spmd(
        _graph(), in_maps, core_ids=list(range(N_CORES)), trace=trace
    )
    LAST_RESULT = res
    R = res.results

    def big(name):
        return np.concatenate(
            [R[c][name].astype(np.float32).transpose(1, 0, 2).reshape(QS, C)
             for c in range(N_CORES)], 0
        )

    def vec(name):
        return np.concatenate([R[c][name].T.reshape(-1) for c in range(N_CORES)])

    p_f, p_r, fused = big("post_f"), big("post_r"), big("fused")
    c_r, c_f = vec("c_r"), vec("c_f")
    w_r, w_f = vec("w_r"), vec("w_f")
    mask = vec("mask")
    n_r = int(round(float(mask.sum())))
    n_f = Q - n_r
    L_f_r = np.array(w_f[:n_f].sum() / c_f.sum(), np.float32)
    L_r_f = np.array(w_r[:n_r].sum() / c_r.sum(), np.float32)
    return (L_f_r, L_r_f, p_f, p_r, fused)


# revision 19
# speedup vs baseline: 1.1293x; 1.1293x over previous
"""Trainium2 Bass kernel for nn_AMFAR (retrieval_knn, 8 NeuronCores).

Strategy: data-parallel over the 65536 queries (8192/core). Host pre-transposes
bf16 queries to [D, Qs] (matmul needs the contraction dim on partitions and
f32 DMA-transpose doesn't exist), precomputes |q|^2 / |p|^2 from the rounded
values and scales protos by -2 so PSUM accumulates the full squared distance.
Device computes sqrt via exp(0.5*ln(x)) to stay inside the one ACT table set
that has both exp and ln. Scalar losses are finished on host from per-query
device outputs (the global n_f cutoff needs a global mask count anyway).
"""
import os
import sys

sys.path.insert(0, "/opt/trn_rl_repo")

import numpy as np
import ml_dtypes

import concourse.bass as bass
import concourse.bacc as bacc
import concourse.tile as tile
from concourse import mybir
from concourse.bass_utils import run_bass_kernel_spmd

BF16 = mybir.dt.bfloat16
F32 = mybir.dt.float32
AF = mybir.ActivationFunctionType
ALU = mybir.AluOpType
AX = mybir.AxisListType

N_CORES = 8
Q, C, D = 65536, 64, 512
QS = Q // N_CORES          # 8192 queries per core
QBLK = 1024                # queries per supertile
NSUP = QS // QBLK          # 8
NT = QBLK // 128           # 8 q-subtiles per supertile
NTILES = QS // 128         # 64 q-subtiles per core
HN = NTILES // 2           # half, for split B/C phases
KCH = D // 128             # 4 contraction chunks
AUGK = 4                   # augmentation rows (q2_hi/q2_lo/1/1), bf16 hi-lo split
MODS = ("r", "f")

LAST_RESULT = None         # BassKernelResults of the last run (for test.py)


def _install_ntff_hook():
    """The agent image's antenv lacks axon_hooks; provide the shim so
    run_bass_kernel_spmd(trace=True) can NTFF-profile through the tunnel."""
    import types

    if "antenv.axon_hooks" in sys.modules:
        return
    mod = types.ModuleType("antenv.axon_hooks")
    mod._hook = None

    def set_axon_ntff_profile_hook(h):
        mod._hook = h

    def get_axon_ntff_profile_hook():
        return mod._hook

    mod.set_axon_ntff_profile_hook = set_axon_ntff_profile_hook
    mod.get_axon_ntff_profile_hook = get_axon_ntff_profile_hook
    sys.modules["antenv.axon_hooks"] = mod
    try:
        from trn_agent_boot.trn_boot import _ntff_profile_via_ctypes

        hook = _ntff_profile_via_ctypes("/opt/axon/libaxon_pjrt.so")
        if hook is not None:
            mod._hook = hook
    except Exception:
        pass


_install_ntff_hook()


def _patch_act_tables():
    """Make natural_log_exp_and_others the only set holding Exp and Ln, so
    bacc's table-load pass emits ONE load instead of thrashing between the
    exp-only and ln-only sets every supertile (33 loads x 1.5us measured)."""
    import functools
    import concourse.hw_specs as hw_specs

    orig = hw_specs.get_activation_tables

    @functools.cache
    def patched(arch):
        out = {}
        for name, fns in orig(arch).items():
            if name != "natural_log_exp_and_others":
                fns = set(fns) - {AF.Exp, AF.Ln}
            out[name] = set(fns)
        return out

    bacc.get_activation_tables = patched


_patch_act_tables()



def _bcast(ap, n):
    """Broadcast an AP along a new innermost free dim of size n (stride 0)."""
    return bass.AP(tensor=ap.tensor, offset=ap.offset, ap=[*ap.ap, [0, n]])


def _flat(ap):
    return ap.rearrange("p a b -> p (a b)")


def _build():
    nc = bacc.Bacc(None, target_bir_lowering=False)
    I, O = {}, {}
    for m in MODS:
        I[f"qT_{m}"] = nc.declare_dram_parameter(f"qT_{m}", [D, QS], BF16, isOutput=False)
        I[f"q2_{m}"] = nc.declare_dram_parameter(f"q2_{m}", [NT * AUGK, NSUP * 128], BF16, isOutput=False)
        I[f"pT_{m}"] = nc.declare_dram_parameter(f"pT_{m}", [128, KCH * C], BF16, isOutput=False)
        I[f"p2_{m}"] = nc.declare_dram_parameter(f"p2_{m}", [NT * AUGK, NT * C], BF16, isOutput=False)
        O[f"post_{m}"] = nc.declare_dram_parameter(f"post_{m}", [128, NTILES, C], BF16, isOutput=True)
        O[f"c_{m}"] = nc.declare_dram_parameter(f"c_{m}", [128, NTILES], F32, isOutput=True)
        O[f"w_{m}"] = nc.declare_dram_parameter(f"w_{m}", [128, NTILES], F32, isOutput=True)
    O["fused"] = nc.declare_dram_parameter("fused", [128, NTILES, C], BF16, isOutput=True)
    O["mask"] = nc.declare_dram_parameter("mask", [128, NTILES], F32, isOutput=True)

    with tile.TileContext(nc) as tc:
        _body(nc, tc, I, O)
    nc.compile()
    return nc


def _body(nc, tc, I, O):
    from contextlib import ExitStack

    with ExitStack() as ctx:
        const = ctx.enter_context(tc.tile_pool(name="const", bufs=1))
        qpool = ctx.enter_context(tc.tile_pool(name="qpool", bufs=3))
        psum = ctx.enter_context(tc.tile_pool(name="psum", bufs=3, space="PSUM"))
        work = ctx.enter_context(tc.tile_pool(name="work", bufs=3))
        outp = ctx.enter_context(tc.tile_pool(name="outp", bufs=3))
        stat = ctx.enter_context(tc.tile_pool(name="stat", bufs=1))
        small = ctx.enter_context(tc.tile_pool(name="small", bufs=2))

        pT, p2 = {}, {}
        for m in MODS:
            pT[m] = const.tile([128, KCH * C], BF16, tag=f"pT_{m}", name=f"pT_{m}")
            nc.sync.dma_start(out=pT[m][:, :], in_=I[f"pT_{m}"][:, :])
            p2[m] = const.tile([NT * AUGK, NT * C], BF16, tag=f"p2_{m}", name=f"p2_{m}")
            nc.sync.dma_start(out=p2[m][:, :], in_=I[f"p2_{m}"][:, :])

        q2all = {}
        for m in MODS:
            q2all[m] = const.tile([NT * AUGK, NSUP * 128], BF16, tag=f"q2a_{m}", name=f"q2a_{m}")
            nc.sync.dma_start(out=q2all[m][:, :], in_=I[f"q2_{m}"][:, :])

        # persistent per-query staging, [128, NTILES(=64)] each
        st = {}
        for nm in ("S_r", "S_f", "rS_r", "rS_f", "ce_r", "ce_f", "pde_r", "pde_f",
                   "Xe", "c_r", "c_f", "w_r", "w_f", "mask", "A"):
            st[nm] = stat.tile([128, NTILES], F32, tag=f"st_{nm}", name=f"st_{nm}")
        # posteriors staged for the whole core (bf16): [128, NTILES, C]
        pall = {}
        for m in MODS:
            pall[m] = stat.tile([128, NTILES, C], BF16, tag=f"pall_{m}", name=f"pall_{m}")

        # ---- phase B+C per half: batched scalar math + fused posterior ----
        def phase_bc(hsl, tag):
            h = {}
            for m in MODS:
                nc.vector.tensor_mul(st[f"c_{m}"][:, hsl], st[f"ce_{m}"][:, hsl], st[f"rS_{m}"][:, hsl])
                lnS = small.tile([128, hsl.stop - hsl.start], F32, tag=f"lnS_{m}{tag}", name=f"lnS_{m}{tag}")
                nc.scalar.activation(lnS[:, :], st[f"S_{m}"][:, hsl], AF.Ln)
                h[m] = small.tile([128, hsl.stop - hsl.start], F32, tag=f"h_{m}{tag}", name=f"h_{m}{tag}")
                nc.vector.tensor_mul(h[m][:, :], st[f"pde_{m}"][:, hsl], st[f"rS_{m}"][:, hsl])
                nc.vector.tensor_add(h[m][:, :], h[m][:, :], lnS[:, :])

            X = small.tile([128, hsl.stop - hsl.start], F32, tag=f"X{tag}", name=f"X{tag}")
            nc.vector.tensor_mul(X[:, :], st["Xe"][:, hsl], st["rS_r"][:, hsl])
            nc.vector.tensor_mul(X[:, :], X[:, :], st["rS_f"][:, hsl])

            for a_, wname in (("f", "w_f"), ("r", "w_r")):
                kl = small.tile([128, hsl.stop - hsl.start], F32, tag=f"kl_{a_}{tag}", name=f"kl_{a_}{tag}")
                nc.vector.tensor_add(kl[:, :], h[a_][:, :], X[:, :])
                nc.vector.tensor_scalar_mul(kl[:, :], kl[:, :], -1.0 / C)
                nc.vector.tensor_mul(st[wname][:, hsl], st[f"c_{a_}"][:, hsl], kl[:, :])

            g1 = small.tile([128, hsl.stop - hsl.start], F32, tag=f"g1{tag}", name=f"g1{tag}")
            nc.vector.tensor_tensor(out=g1[:, :], in0=h["r"][:, :], in1=h["f"][:, :], op=ALU.is_gt)
            g2 = small.tile([128, hsl.stop - hsl.start], F32, tag=f"g2{tag}", name=f"g2{tag}")
            nc.vector.tensor_tensor(out=g2[:, :], in0=st["c_r"][:, hsl], in1=st["c_f"][:, hsl], op=ALU.is_gt)
            nc.vector.tensor_mul(st["mask"][:, hsl], g1[:, :], g2[:, :])

            den = small.tile([128, hsl.stop - hsl.start], F32, tag=f"den{tag}", name=f"den{tag}")
            nc.vector.tensor_add(den[:, :], st["c_r"][:, hsl], st["c_f"][:, hsl])
            rden = small.tile([128, hsl.stop - hsl.start], F32, tag=f"rden{tag}", name=f"rden{tag}")
            nc.vector.reciprocal(rden[:, :], den[:, :])
            wr = small.tile([128, hsl.stop - hsl.start], F32, tag=f"wr{tag}", name=f"wr{tag}")
            nc.vector.tensor_mul(wr[:, :], st["c_r"][:, hsl], rden[:, :])
            a_t = small.tile([128, hsl.stop - hsl.start], F32, tag=f"a_t{tag}", name=f"a_t{tag}")
            nc.vector.tensor_mul(a_t[:, :], wr[:, :], st["S_r"][:, hsl])
            wf = small.tile([128, hsl.stop - hsl.start], F32, tag=f"wf{tag}", name=f"wf{tag}")
            nc.vector.tensor_scalar(wf[:, :], wr[:, :], -1.0, 1.0, ALU.mult, ALU.add)
            b_t = small.tile([128, hsl.stop - hsl.start], F32, tag=f"b_t{tag}", name=f"b_t{tag}")
            nc.vector.tensor_mul(b_t[:, :], wf[:, :], st["S_f"][:, hsl])
            T_t = small.tile([128, hsl.stop - hsl.start], F32, tag=f"T_t{tag}", name=f"T_t{tag}")
            nc.vector.tensor_add(T_t[:, :], a_t[:, :], b_t[:, :])
            rT = small.tile([128, hsl.stop - hsl.start], F32, tag=f"rT{tag}", name=f"rT{tag}")
            nc.vector.reciprocal(rT[:, :], T_t[:, :])
            A_t = small.tile([128, hsl.stop - hsl.start], BF16, tag=f"A_t{tag}", name=f"A_t{tag}")
            nc.vector.tensor_mul(A_t[:, :], a_t[:, :], rT[:, :])

            # fused = A*(p_r - p_f) + p_f
            for su in range(hsl.start // NT, hsl.stop // NT):
                ssl = slice(su * NT, (su + 1) * NT)
                asl = slice(su * NT - hsl.start, (su + 1) * NT - hsl.start)
                df = work.tile([128, NT, C], BF16, tag="t1", name="t1")
                nc.vector.tensor_sub(df[:, :, :], pall["r"][:, ssl, :], pall["f"][:, ssl, :])
                t2 = work.tile([128, NT, C], BF16, tag="t2", name="t2")
                nc.vector.tensor_tensor(
                    out=t2[:, :, :], in0=df[:, :, :], in1=_bcast(A_t[:, asl], C),
                    op=ALU.mult,
                )
                fu = outp.tile([128, NT, C], BF16, tag="fused", name="fused")
                nc.vector.tensor_add(fu[:, :, :], t2[:, :, :], pall["f"][:, ssl, :])
                nc.scalar.dma_start(out=O["fused"][:, ssl, :], in_=fu[:, :, :])


        # ---- phase A: distances, posteriors, raw stats ----
        for su in range(NSUP):
            q0 = su * QBLK
            ssl = slice(su * NT, (su + 1) * NT)

            qt = {}
            for m in MODS:
                for k in range(KCH):
                    t = qpool.tile([128, QBLK], BF16, tag=f"qt_{m}{k}", name=f"qt_{m}{k}")
                    nc.sync.dma_start(
                        out=t[:, :], in_=I[f"qT_{m}"][k * 128:(k + 1) * 128, q0:q0 + QBLK]
                    )
                    qt[m, k] = t

            ps = {}
            for m in MODS:
                P_ = psum.tile([128, NT, C], F32, tag=f"ps_{m}", name=f"ps_{m}")
                nc.tensor.matmul(
                    _flat(P_),
                    lhsT=q2all[m][:, su * 128:(su + 1) * 128],
                    rhs=p2[m][:, :],
                    start=True,
                    stop=False,
                )
                for t_i in range(NT):
                    sl = slice(t_i * 128, (t_i + 1) * 128)
                    for k in range(KCH):
                        nc.tensor.matmul(
                            P_[:, t_i, :],
                            lhsT=qt[m, k][:, sl],
                            rhs=pT[m][:, k * C:(k + 1) * C],
                            start=False,
                            stop=(k == KCH - 1),
                            skip_group_check=True,
                        )
                ps[m] = P_

            e, d = {}, {}
            for m in MODS:
                lnt = work.tile([128, NT * C], F32, tag=f"ln_{m}", name=f"ln_{m}")
                nc.scalar.activation(lnt[:, :], _flat(ps[m]), AF.Ln)
                d[m] = work.tile([128, NT, C], F32, tag=f"d_{m}", name=f"d_{m}")
                nc.scalar.activation(_flat(d[m]), lnt[:, :], AF.Exp, scale=0.5)
                e[m] = work.tile([128, NT, C], BF16, tag=f"e_{m}", name=f"e_{m}")
                nc.scalar.activation(_flat(e[m]), _flat(d[m]), AF.Exp, scale=-1.0)

                nc.vector.reduce_sum(st[f"S_{m}"][:, ssl], e[m][:, :, :], axis=AX.X)
                nc.vector.reciprocal(st[f"rS_{m}"][:, ssl], st[f"S_{m}"][:, ssl])
                nc.vector.reduce_max(st[f"ce_{m}"][:, ssl], e[m][:, :, :], axis=AX.X)
                rSb = small.tile([128, NT], BF16, tag=f"rSb_{m}", name=f"rSb_{m}")
                nc.vector.tensor_copy(rSb[:, :], st[f"rS_{m}"][:, ssl])
                # posterior p = e * rS  (bf16, staged + output)
                pv = pall[m][:, ssl, :]
                nc.vector.tensor_tensor(
                    out=pv, in0=e[m][:, :, :], in1=_bcast(rSb[:, :], C),
                    op=ALU.mult,
                )
                nc.scalar.dma_start(out=O[f"post_{m}"][:, ssl, :], in_=pv)

                # sum(e*d) for entropy
                pdt = work.tile([128, NT, C], F32, tag="scratch", name="scratch")
                nc.gpsimd.tensor_mul(pdt[:, :, :], e[m][:, :, :], d[m][:, :, :])
                nc.vector.reduce_sum(st[f"pde_{m}"][:, ssl], pdt[:, :, :], axis=AX.X)

            # cross term sum(e_r*e_f)
            xt = work.tile([128, NT, C], BF16, tag="xscr", name="xscr")
            nc.vector.tensor_mul(xt[:, :, :], e["r"][:, :, :], e["f"][:, :, :])
            nc.vector.reduce_sum(st["Xe"][:, ssl], xt[:, :, :], axis=AX.X)

            if su == NSUP - 1:
                phase_bc(slice(0, NSUP * NT), "a")

        for nm in ("c_r", "c_f", "w_r", "w_f", "mask"):
            nc.scalar.dma_start(out=O[nm][:, :], in_=st[nm][:, :])


_GRAPH = None


def _graph():
    global _GRAPH
    if _GRAPH is None:
        _GRAPH = _build()
    return _GRAPH


def kernel(context_rgb_features, context_flow_features,
           target_rgb_features, target_flow_features):
    global LAST_RESULT
    bf = ml_dtypes.bfloat16
    ctxf = {"r": np.asarray(context_rgb_features), "f": np.asarray(context_flow_features)}
    tgt = {"r": np.asarray(target_rgb_features), "f": np.asarray(target_flow_features)}

    pT_in, p2_in = {}, {}
    for m in MODS:
        pb = ctxf[m].astype(bf)                                   # [C, D] rounded
        pbf = pb.astype(np.float32)
        pTs = np.ascontiguousarray((-2.0 * pbf).astype(bf).T)     # [D, C] = -2*p~
        pT_in[m] = np.ascontiguousarray(
            pTs.reshape(KCH, 128, C).transpose(1, 0, 2)
        ).reshape(128, KCH * C)
        p2v = (pbf * pbf).sum(1)
        p2_hi = p2v.astype(bf).astype(np.float32)
        p2row = np.zeros((AUGK, C), bf)
        p2row[0] = 1.0                                            # pairs q2_hi row
        p2row[1] = 1.0                                            # pairs q2_lo row
        p2row[2] = p2_hi.astype(bf)                               # pairs ones row
        p2row[3] = (p2v - p2_hi).astype(bf)                       # residual
        # block-diagonal over the NT subtiles: rows (t,j), cols (t,c)
        p2a = np.zeros((NT * AUGK, NT * C), bf)
        for t in range(NT):
            p2a[t * AUGK:(t + 1) * AUGK, t * C:(t + 1) * C] = p2row
        p2_in[m] = p2a

    in_maps = []
    for core in range(N_CORES):
        sl = slice(core * QS, (core + 1) * QS)
        im = {}
        for m in MODS:
            tb = tgt[m][sl].astype(bf)                            # [QS, D] rounded
            tf32 = tb.astype(np.float32)
            im[f"qT_{m}"] = np.ascontiguousarray(tb.T)            # [D, QS]
            q2v = np.einsum("qd,qd->q", tf32, tf32)
            q2_hi = q2v.astype(bf).astype(np.float32)
            q2lo = (q2v - q2_hi)
            # rows (t,j), cols (su,p): value = aug_j(su*QBLK + t*128 + p)
            q2a = np.empty((NT, AUGK, NSUP, 128), bf)
            qq = q2_hi.reshape(NSUP, NT, 128)
            ql = q2lo.reshape(NSUP, NT, 128)
            q2a[:, 0] = qq.transpose(1, 0, 2).astype(bf)
            q2a[:, 1] = ql.transpose(1, 0, 2).astype(bf)
            q2a[:, 2] = 1.0
            q2a[:, 3] = 1.0
            im[f"q2_{m}"] = q2a.reshape(NT * AUGK, NSUP * 128)
            im[f"pT_{m}"] = pT_in[m]
            im[f"p2_{m}"] = p2_in[m]
        in_maps.append(im)

    trace = os.environ.get("KERNEL_TRACE", "0") == "1"
    res = run_bass_kernel_spmd(
        _graph(), in_maps, core_ids=list(range(N_CORES)), trace=trace
    )
    LAST_RESULT = res
    R = res.results

    def big(name):
        return np.concatenate(
            [R[c][name].astype(np.float32).transpose(1, 0, 2).reshape(QS, C)
             for c in range(N_CORES)], 0
        )

    def vec(name):
        return np.concatenate([R[c][name].T.reshape(-1) for c in range(N_CORES)])

    p_f, p_r, fused = big("post_f"), big("post_r"), big("fused")
    c_r, c_f = vec("c_r"), vec("c_f")
    w_r, w_f = vec("w_r"), vec("w_f")
    mask = vec("mask")
    n_r = int(round(float(mask.sum())))
    n_f = Q - n_r
    L_f_r = np.array(w_f[:n_f].sum() / c_f.sum(), np.float32)
    L_r_f = np.array(w_r[:n_r].sum() / c_r.sum(), np.float32)
    return (L_f_r, L_r_f, p_f, p_r, fused)


# revision 21
# speedup vs baseline: 1.2068x; 1.0686x over previous
"""Trainium2 Bass kernel for nn_AMFAR (retrieval_knn, 8 NeuronCores).

Strategy: data-parallel over the 65536 queries (8192/core). Host pre-transposes
bf16 queries to [D, Qs] (matmul needs the contraction dim on partitions and
f32 DMA-transpose doesn't exist), precomputes |q|^2 / |p|^2 from the rounded
values and scales protos by -2 so PSUM accumulates the full squared distance.
Device computes sqrt via exp(0.5*ln(x)) to stay inside the one ACT table set
that has both exp and ln. Scalar losses are finished on host from per-query
device outputs (the global n_f cutoff needs a global mask count anyway).
"""
import os
import sys

sys.path.insert(0, "/opt/trn_rl_repo")

import numpy as np
import ml_dtypes

import concourse.bass as bass
import concourse.bacc as bacc
import concourse.tile as tile
from concourse import mybir
from concourse.bass_utils import run_bass_kernel_spmd

BF16 = mybir.dt.bfloat16
F32 = mybir.dt.float32
AF = mybir.ActivationFunctionType
ALU = mybir.AluOpType
AX = mybir.AxisListType

N_CORES = 8
Q, C, D = 65536, 64, 512
QS = Q // N_CORES          # 8192 queries per core
QBLK = 2048                # queries per supertile
NSUP = QS // QBLK          # 4
NT = QBLK // 128           # 16 q-subtiles per supertile
NTILES = QS // 128         # 64 q-subtiles per core
HN = NTILES // 2           # half, for split B/C phases
KCH = D // 128             # 4 contraction chunks
AUGK = 4                   # augmentation rows (q2_hi/q2_lo/1/1), bf16 hi-lo split
MODS = ("r", "f")

LAST_RESULT = None         # BassKernelResults of the last run (for test.py)


def _install_ntff_hook():
    """The agent image's antenv lacks axon_hooks; provide the shim so
    run_bass_kernel_spmd(trace=True) can NTFF-profile through the tunnel."""
    import types

    if "antenv.axon_hooks" in sys.modules:
        return
    mod = types.ModuleType("antenv.axon_hooks")
    mod._hook = None

    def set_axon_ntff_profile_hook(h):
        mod._hook = h

    def get_axon_ntff_profile_hook():
        return mod._hook

    mod.set_axon_ntff_profile_hook = set_axon_ntff_profile_hook
    mod.get_axon_ntff_profile_hook = get_axon_ntff_profile_hook
    sys.modules["antenv.axon_hooks"] = mod
    try:
        from trn_agent_boot.trn_boot import _ntff_profile_via_ctypes

        hook = _ntff_profile_via_ctypes("/opt/axon/libaxon_pjrt.so")
        if hook is not None:
            mod._hook = hook
    except Exception:
        pass


_install_ntff_hook()


def _patch_act_tables():
    """Make natural_log_exp_and_others the only set holding Exp and Ln, so
    bacc's table-load pass emits ONE load instead of thrashing between the
    exp-only and ln-only sets every supertile (33 loads x 1.5us measured)."""
    import functools
    import concourse.hw_specs as hw_specs

    orig = hw_specs.get_activation_tables

    @functools.cache
    def patched(arch):
        out = {}
        for name, fns in orig(arch).items():
            if name != "natural_log_exp_and_others":
                fns = set(fns) - {AF.Exp, AF.Ln}
            out[name] = set(fns)
        return out

    bacc.get_activation_tables = patched


_patch_act_tables()



def _bcast(ap, n):
    """Broadcast an AP along a new innermost free dim of size n (stride 0)."""
    return bass.AP(tensor=ap.tensor, offset=ap.offset, ap=[*ap.ap, [0, n]])


def _flat(ap):
    return ap.rearrange("p a b -> p (a b)")


def _build():
    nc = bacc.Bacc(None, target_bir_lowering=False)
    I, O = {}, {}
    for m in MODS:
        I[f"qT_{m}"] = nc.declare_dram_parameter(f"qT_{m}", [D, QS], BF16, isOutput=False)
        I[f"q2_{m}"] = nc.declare_dram_parameter(f"q2_{m}", [8 * AUGK, (NT // 8) * NSUP * 128], BF16, isOutput=False)
        I[f"pT_{m}"] = nc.declare_dram_parameter(f"pT_{m}", [128, KCH * C], BF16, isOutput=False)
        I[f"p2_{m}"] = nc.declare_dram_parameter(f"p2_{m}", [8 * AUGK, 8 * C], BF16, isOutput=False)
        O[f"post_{m}"] = nc.declare_dram_parameter(f"post_{m}", [128, NTILES, C], BF16, isOutput=True)
        O[f"c_{m}"] = nc.declare_dram_parameter(f"c_{m}", [128, NTILES], F32, isOutput=True)
        O[f"w_{m}"] = nc.declare_dram_parameter(f"w_{m}", [128, NTILES], F32, isOutput=True)
    O["fused"] = nc.declare_dram_parameter("fused", [128, NTILES, C], BF16, isOutput=True)
    O["mask"] = nc.declare_dram_parameter("mask", [128, NTILES], F32, isOutput=True)

    with tile.TileContext(nc) as tc:
        _body(nc, tc, I, O)
    nc.compile()
    return nc


def _body(nc, tc, I, O):
    from contextlib import ExitStack

    with ExitStack() as ctx:
        const = ctx.enter_context(tc.tile_pool(name="const", bufs=1))
        qpool = ctx.enter_context(tc.tile_pool(name="qpool", bufs=2))
        psum = ctx.enter_context(tc.tile_pool(name="psum", bufs=2, space="PSUM"))
        work = ctx.enter_context(tc.tile_pool(name="work", bufs=2))
        outp = ctx.enter_context(tc.tile_pool(name="outp", bufs=2))
        stat = ctx.enter_context(tc.tile_pool(name="stat", bufs=1))
        small = ctx.enter_context(tc.tile_pool(name="small", bufs=2))

        pT, p2 = {}, {}
        for m in MODS:
            pT[m] = const.tile([128, KCH * C], BF16, tag=f"pT_{m}", name=f"pT_{m}")
            nc.sync.dma_start(out=pT[m][:, :], in_=I[f"pT_{m}"][:, :])
            p2[m] = const.tile([8 * AUGK, 8 * C], BF16, tag=f"p2_{m}", name=f"p2_{m}")
            nc.sync.dma_start(out=p2[m][:, :], in_=I[f"p2_{m}"][:, :])

        q2all = {}
        for m in MODS:
            q2all[m] = const.tile([8 * AUGK, (NT // 8) * NSUP * 128], BF16, tag=f"q2a_{m}", name=f"q2a_{m}")
            nc.sync.dma_start(out=q2all[m][:, :], in_=I[f"q2_{m}"][:, :])

        # persistent per-query staging, [128, NTILES(=64)] each
        st = {}
        for nm in ("S_r", "S_f", "rS_r", "rS_f", "ce_r", "ce_f", "pde_r", "pde_f",
                   "Xe", "c_r", "c_f", "w_r", "w_f", "mask", "A"):
            st[nm] = stat.tile([128, NTILES], F32, tag=f"st_{nm}", name=f"st_{nm}")
        # posteriors staged for the whole core (bf16): [128, NTILES, C]
        pall = {}
        for m in MODS:
            pall[m] = stat.tile([128, NTILES, C], BF16, tag=f"pall_{m}", name=f"pall_{m}")

        # ---- phase B+C per half: batched scalar math + fused posterior ----
        def phase_bc(hsl, tag):
            h = {}
            for m in MODS:
                nc.vector.tensor_mul(st[f"c_{m}"][:, hsl], st[f"ce_{m}"][:, hsl], st[f"rS_{m}"][:, hsl])
                lnS = small.tile([128, hsl.stop - hsl.start], F32, tag=f"lnS_{m}{tag}", name=f"lnS_{m}{tag}")
                nc.scalar.activation(lnS[:, :], st[f"S_{m}"][:, hsl], AF.Ln)
                h[m] = small.tile([128, hsl.stop - hsl.start], F32, tag=f"h_{m}{tag}", name=f"h_{m}{tag}")
                nc.vector.tensor_mul(h[m][:, :], st[f"pde_{m}"][:, hsl], st[f"rS_{m}"][:, hsl])
                nc.vector.tensor_add(h[m][:, :], h[m][:, :], lnS[:, :])

            X = small.tile([128, hsl.stop - hsl.start], F32, tag=f"X{tag}", name=f"X{tag}")
            nc.vector.tensor_mul(X[:, :], st["Xe"][:, hsl], st["rS_r"][:, hsl])
            nc.vector.tensor_mul(X[:, :], X[:, :], st["rS_f"][:, hsl])

            for a_, wname in (("f", "w_f"), ("r", "w_r")):
                kl = small.tile([128, hsl.stop - hsl.start], F32, tag=f"kl_{a_}{tag}", name=f"kl_{a_}{tag}")
                nc.vector.tensor_add(kl[:, :], h[a_][:, :], X[:, :])
                nc.vector.tensor_scalar_mul(kl[:, :], kl[:, :], -1.0 / C)
                nc.vector.tensor_mul(st[wname][:, hsl], st[f"c_{a_}"][:, hsl], kl[:, :])

            g1 = small.tile([128, hsl.stop - hsl.start], F32, tag=f"g1{tag}", name=f"g1{tag}")
            nc.vector.tensor_tensor(out=g1[:, :], in0=h["r"][:, :], in1=h["f"][:, :], op=ALU.is_gt)
            g2 = small.tile([128, hsl.stop - hsl.start], F32, tag=f"g2{tag}", name=f"g2{tag}")
            nc.vector.tensor_tensor(out=g2[:, :], in0=st["c_r"][:, hsl], in1=st["c_f"][:, hsl], op=ALU.is_gt)
            nc.vector.tensor_mul(st["mask"][:, hsl], g1[:, :], g2[:, :])

            den = small.tile([128, hsl.stop - hsl.start], F32, tag=f"den{tag}", name=f"den{tag}")
            nc.vector.tensor_add(den[:, :], st["c_r"][:, hsl], st["c_f"][:, hsl])
            rden = small.tile([128, hsl.stop - hsl.start], F32, tag=f"rden{tag}", name=f"rden{tag}")
            nc.vector.reciprocal(rden[:, :], den[:, :])
            wr = small.tile([128, hsl.stop - hsl.start], F32, tag=f"wr{tag}", name=f"wr{tag}")
            nc.vector.tensor_mul(wr[:, :], st["c_r"][:, hsl], rden[:, :])
            a_t = small.tile([128, hsl.stop - hsl.start], F32, tag=f"a_t{tag}", name=f"a_t{tag}")
            nc.vector.tensor_mul(a_t[:, :], wr[:, :], st["S_r"][:, hsl])
            wf = small.tile([128, hsl.stop - hsl.start], F32, tag=f"wf{tag}", name=f"wf{tag}")
            nc.vector.tensor_scalar(wf[:, :], wr[:, :], -1.0, 1.0, ALU.mult, ALU.add)
            b_t = small.tile([128, hsl.stop - hsl.start], F32, tag=f"b_t{tag}", name=f"b_t{tag}")
            nc.vector.tensor_mul(b_t[:, :], wf[:, :], st["S_f"][:, hsl])
            T_t = small.tile([128, hsl.stop - hsl.start], F32, tag=f"T_t{tag}", name=f"T_t{tag}")
            nc.vector.tensor_add(T_t[:, :], a_t[:, :], b_t[:, :])
            rT = small.tile([128, hsl.stop - hsl.start], F32, tag=f"rT{tag}", name=f"rT{tag}")
            nc.vector.reciprocal(rT[:, :], T_t[:, :])
            A_t = small.tile([128, hsl.stop - hsl.start], BF16, tag=f"A_t{tag}", name=f"A_t{tag}")
            nc.vector.tensor_mul(A_t[:, :], a_t[:, :], rT[:, :])

            # fused = A*(p_r - p_f) + p_f
            for su in range(hsl.start // NT, hsl.stop // NT):
                ssl = slice(su * NT, (su + 1) * NT)
                asl = slice(su * NT - hsl.start, (su + 1) * NT - hsl.start)
                df = work.tile([128, NT, C], BF16, tag="t1", name="t1")
                nc.vector.tensor_sub(df[:, :, :], pall["r"][:, ssl, :], pall["f"][:, ssl, :])
                t2 = work.tile([128, NT, C], BF16, tag="t2", name="t2")
                nc.vector.tensor_tensor(
                    out=t2[:, :, :], in0=df[:, :, :], in1=_bcast(A_t[:, asl], C),
                    op=ALU.mult,
                )
                fu = outp.tile([128, NT, C], BF16, tag="fused", name="fused")
                nc.vector.tensor_add(fu[:, :, :], t2[:, :, :], pall["f"][:, ssl, :])
                nc.scalar.dma_start(out=O["fused"][:, ssl, :], in_=fu[:, :, :])


        # ---- phase A: distances, posteriors, raw stats ----
        for su in range(NSUP):
            q0 = su * QBLK
            ssl = slice(su * NT, (su + 1) * NT)

            qt = {}
            for m in MODS:
                for k in range(KCH):
                    t = qpool.tile([128, QBLK], BF16, tag=f"qt_{m}{k}", name=f"qt_{m}{k}")
                    nc.sync.dma_start(
                        out=t[:, :], in_=I[f"qT_{m}"][k * 128:(k + 1) * 128, q0:q0 + QBLK]
                    )
                    qt[m, k] = t

            ps = {}
            for m in MODS:
                P_ = psum.tile([128, NT, C], F32, tag=f"ps_{m}", name=f"ps_{m}")
                fl = _flat(P_)
                for hh in range(NT // 8):
                    nc.tensor.matmul(
                        fl[:, hh * 8 * C:(hh + 1) * 8 * C],
                        lhsT=q2all[m][:, (hh * NSUP + su) * 128:(hh * NSUP + su + 1) * 128],
                        rhs=p2[m][:, :],
                        start=True,
                        stop=False,
                    )
                for t_i in range(NT):
                    sl = slice(t_i * 128, (t_i + 1) * 128)
                    for k in range(KCH):
                        nc.tensor.matmul(
                            P_[:, t_i, :],
                            lhsT=qt[m, k][:, sl],
                            rhs=pT[m][:, k * C:(k + 1) * C],
                            start=False,
                            stop=(k == KCH - 1),
                            skip_group_check=True,
                        )
                ps[m] = P_

            e, d = {}, {}
            for m in MODS:
                lnt = work.tile([128, NT * C], F32, tag=f"ln_{m}", name=f"ln_{m}")
                nc.scalar.activation(lnt[:, :], _flat(ps[m]), AF.Ln)
                d[m] = work.tile([128, NT, C], F32, tag=f"d_{m}", name=f"d_{m}")
                nc.scalar.activation(_flat(d[m]), lnt[:, :], AF.Exp, scale=0.5)
                e[m] = work.tile([128, NT, C], BF16, tag=f"e_{m}", name=f"e_{m}")
                nc.scalar.activation(_flat(e[m]), _flat(d[m]), AF.Exp, scale=-1.0)

                nc.vector.reduce_sum(st[f"S_{m}"][:, ssl], e[m][:, :, :], axis=AX.X)
                nc.vector.reciprocal(st[f"rS_{m}"][:, ssl], st[f"S_{m}"][:, ssl])
                nc.vector.reduce_max(st[f"ce_{m}"][:, ssl], e[m][:, :, :], axis=AX.X)
                rSb = small.tile([128, NT], BF16, tag=f"rSb_{m}", name=f"rSb_{m}")
                nc.vector.tensor_copy(rSb[:, :], st[f"rS_{m}"][:, ssl])
                # posterior p = e * rS  (bf16, staged + output)
                pv = pall[m][:, ssl, :]
                nc.vector.tensor_tensor(
                    out=pv, in0=e[m][:, :, :], in1=_bcast(rSb[:, :], C),
                    op=ALU.mult,
                )
                nc.scalar.dma_start(out=O[f"post_{m}"][:, ssl, :], in_=pv)

                # sum(e*d) for entropy
                pdt = work.tile([128, NT, C], F32, tag="scratch", name="scratch")
                nc.gpsimd.tensor_mul(pdt[:, :, :], e[m][:, :, :], d[m][:, :, :])
                nc.vector.reduce_sum(st[f"pde_{m}"][:, ssl], pdt[:, :, :], axis=AX.X)

            # cross term sum(e_r*e_f)
            xt = work.tile([128, NT, C], BF16, tag="xscr", name="xscr")
            nc.vector.tensor_mul(_flat(xt), _flat(e["r"]), _flat(e["f"]))
            nc.vector.reduce_sum(st["Xe"][:, ssl], xt[:, :, :], axis=AX.X)

            if su == NSUP - 1:
                phase_bc(slice(0, NSUP * NT), "a")

        for nm in ("c_r", "c_f", "w_r", "w_f", "mask"):
            nc.scalar.dma_start(out=O[nm][:, :], in_=st[nm][:, :])


_GRAPH = None


def _graph():
    global _GRAPH
    if _GRAPH is None:
        _GRAPH = _build()
    return _GRAPH


def kernel(context_rgb_features, context_flow_features,
           target_rgb_features, target_flow_features):
    global LAST_RESULT
    bf = ml_dtypes.bfloat16
    ctxf = {"r": np.asarray(context_rgb_features), "f": np.asarray(context_flow_features)}
    tgt = {"r": np.asarray(target_rgb_features), "f": np.asarray(target_flow_features)}

    pT_in, p2_in = {}, {}
    for m in MODS:
        pb = ctxf[m].astype(bf)                                   # [C, D] rounded
        pbf = pb.astype(np.float32)
        pTs = np.ascontiguousarray((-2.0 * pbf).astype(bf).T)     # [D, C] = -2*p~
        pT_in[m] = np.ascontiguousarray(
            pTs.reshape(KCH, 128, C).transpose(1, 0, 2)
        ).reshape(128, KCH * C)
        p2v = (pbf * pbf).sum(1)
        p2_hi = p2v.astype(bf).astype(np.float32)
        p2row = np.zeros((AUGK, C), bf)
        p2row[0] = 1.0                                            # pairs q2_hi row
        p2row[1] = 1.0                                            # pairs q2_lo row
        p2row[2] = p2_hi.astype(bf)                               # pairs ones row
        p2row[3] = (p2v - p2_hi).astype(bf)                       # residual
        # block-diagonal over the NT subtiles: rows (t,j), cols (t,c)
        p2a = np.zeros((8 * AUGK, 8 * C), bf)
        for t in range(8):
            p2a[t * AUGK:(t + 1) * AUGK, t * C:(t + 1) * C] = p2row
        p2_in[m] = p2a

    in_maps = []
    for core in range(N_CORES):
        sl = slice(core * QS, (core + 1) * QS)
        im = {}
        for m in MODS:
            tb = tgt[m][sl].astype(bf)                            # [QS, D] rounded
            tf32 = tb.astype(np.float32)
            im[f"qT_{m}"] = np.ascontiguousarray(tb.T)            # [D, QS]
            q2v = np.einsum("qd,qd->q", tf32, tf32)
            q2_hi = q2v.astype(bf).astype(np.float32)
            q2lo = (q2v - q2_hi)
            # rows ((t%8),j), cols (t//8, su, p): value = aug_j(su*QBLK + t*128 + p)
            NH = NT // 8
            q2a = np.empty((8, AUGK, NH, NSUP, 128), bf)
            qq = q2_hi.reshape(NSUP, NH, 8, 128)     # [su, h, t', p]
            ql = q2lo.reshape(NSUP, NH, 8, 128)
            q2a[:, 0] = qq.transpose(2, 1, 0, 3).astype(bf)
            q2a[:, 1] = ql.transpose(2, 1, 0, 3).astype(bf)
            q2a[:, 2] = 1.0
            q2a[:, 3] = 1.0
            im[f"q2_{m}"] = q2a.reshape(8 * AUGK, NH * NSUP * 128)
            im[f"pT_{m}"] = pT_in[m]
            im[f"p2_{m}"] = p2_in[m]
        in_maps.append(im)

    trace = os.environ.get("KERNEL_TRACE", "0") == "1"
    res = run_bass_kernel_spmd(
        _graph(), in_maps, core_ids=list(range(N_CORES)), trace=trace
    )
    LAST_RESULT = res
    R = res.results

    def big(name):
        return np.concatenate(
            [R[c][name].astype(np.float32).transpose(1, 0, 2).reshape(QS, C)
             for c in range(N_CORES)], 0
        )

    def vec(name):
        return np.concatenate([R[c][name].T.reshape(-1) for c in range(N_CORES)])

    p_f, p_r, fused = big("post_f"), big("post_r"), big("fused")
    c_r, c_f = vec("c_r"), vec("c_f")
    w_r, w_f = vec("w_r"), vec("w_f")
    mask = vec("mask")
    n_r = int(round(float(mask.sum())))
    n_f = Q - n_r
    L_f_r = np.array(w_f[:n_f].sum() / c_f.sum(), np.float32)
    L_r_f = np.array(w_r[:n_r].sum() / c_r.sum(), np.float32)
    return (L_f_r, L_r_f, p_f, p_r, fused)
